# revision 54
# baseline (speedup 1.0000x reference)
"""MRU encoding kernel for Trainium2 (8 NeuronCores, batch-parallel).

Problem (B=32, T=2048, D=300):
    z = tanh(x @ Wz.T + bz); o = tanh(x @ Wo.T + bo)
    c_t = g_t*c_{t-1} + (1-g_t)*z_t   (c_{-1}=0, scan over T)
    out = o * c

Per-core (4 batch rows) layout is [channel, time]:
  - host pre-transposes x,g to [b, D, T]; x gets a ones-row (301) so the
    bias rides in the matmul contraction; the two weight matrices are fed
    as ONE combined [D+1, 5, 128] operand: slices 0,1 = Wz.T columns
    0:256, slices 2,3 = Wo.T columns 0:256, slice 4 = the ragged columns
    of BOTH weights ([Wz.T 256:300 | zeros | Wo.T 256:300 | zeros]) so
    one GEMM per (row, slice) covers them: 15 matmul groups per row
    instead of 18.
  - o is produced NEGATED via tanh(scale=-1): with bneg=(g-1)*z = -(1-g)z
    the hardware scan state=g*state+bneg yields -c, and (-o)*(-c) = o*c.
  - the whole T=2048 recurrence per channel is ONE tensor_tensor_scan
    DVE instruction per 128-channel tile (state kept fp32 by HW); the
    final res mult rides the otherwise-idle GPSIMD engine (its only
    consumer is a deferred store, so GPSIMD's latency is free).
  - psums are half-T [128,1024] tiles in a strict 4-deep round-robin so
    a GEMM's WAR wait lands on an activation from a full unit earlier.
  - the ragged slice-4 psum holds z at partitions 0:44 and o at 64:108;
    partition-shifted activations repack BOTH batch rows of a pair into
    one 128-lane z2/oneg2 pair (b_even at 0:44, b_odd at 64:108) so the
    ragged recurrence is one DVE chain per pair.  The zero weight-pad
    columns make the activations also rewrite the pad lanes (tanh(0)=0),
    keeping every lane finite without extra memsets.
  - input loads ride the SP HWDGE ring; chain stores also ride SP but are
    deferred past the next pair's loads (a pending store holds the
    issuing SEQ, so it must never sit ahead of latency-critical work);
    weights ride the ACT ring; a 2-matmul warm-up puts the PE p-state
    ramp before the first real GEMM.
"""

import numpy as np

import concourse.bass as bass
import concourse.mybir as mybir
import concourse.tile as tile
from concourse import bacc
from concourse.bass_utils import run_bass_kernel_spmd

B, T, D = 32, 2048, 300
NCORES = 8
BC = B // NCORES  # 4 batch rows per core
DP = D + 1  # ones-row at index 300 carries the bias
NS = 5  # combined-weight m-slices: z0 z1 o0 o1 ragged
TS = 512  # moving-operand max free dim
NT = T // TS
F32 = mybir.dt.float32
F32R = mybir.dt.float32r
F16 = mybir.dt.float16

KC = [(0, 128), (128, 128), (256, 45)]  # k-chunks (incl. ones row)

CFG = {"mm16": True, "plane16": True, "c16": True, "out16": True}

_CACHE: dict = {}

Tanh = mybir.ActivationFunctionType.Tanh


def _build_program(reps=1, bufs=None, cfg=None):
    c = dict(CFG)
    if cfg:
        c.update(cfg)
    mm_dt = F16 if c["mm16"] else F32R
    pl_dt = F16 if c["plane16"] else F32
    c_dt = F16 if c["c16"] else F32
    out_dt = F16 if c["out16"] else F32

    bf = {"xp": 2, "gp": 2, "zp": 2, "ep": 4, "ps": 1}
    if bufs:
        bf.update(bufs)

    nc = bacc.Bacc("TRN2", target_bir_lowering=False, debug=False, num_devices=NCORES)

    d_x = nc.dram_tensor("xt", [BC, DP, T], mm_dt, kind="ExternalInput").ap()
    d_g = nc.dram_tensor("gt", [BC, D, T], pl_dt, kind="ExternalInput").ap()
    d_w = nc.dram_tensor("wzo", [DP, NS * 128], mm_dt, kind="ExternalInput").ap()
    # replicas share ONE output tensor: keeps the PJRT buffer count (and its
    # per-call overhead) constant across reps so marginal timing is clean
    d_out0 = nc.dram_tensor("outt", [BC, D, T], out_dt, kind="ExternalOutput").ap()
    d_outs = [d_out0] * reps

    with tile.TileContext(nc) as tc:
        with (
            tc.tile_pool(name="wp", bufs=1) as wp,
            tc.tile_pool(name="g2p", bufs=1) as g2p,
            tc.tile_pool(name="xp", bufs=bf["xp"]) as xp,
            tc.tile_pool(name="gp", bufs=bf["gp"]) as gp,
            tc.tile_pool(name="zp", bufs=bf["zp"]) as zp,
            tc.tile_pool(name="ep", bufs=bf["ep"]) as ep,
            tc.tile_pool(name="ps", bufs=bf["ps"], space="PSUM") as ps,
        ):
            # weights ride the scalar ring so they don't delay the first x
            # load; the first matmul's chunk (k-chunk 0, slice 0) goes out
            # alone so the opening GEMM isn't gated on the bulk transfer
            w = wp.tile([128, 3, NS, 128], mm_dt, tag="w", name="w_t")
            nc.scalar.dma_start(w[:, 0, 0, :], d_w[0:128, 0:128])
            nc.scalar.dma_start(
                w[:, 0, 1:NS, :],
                d_w[0:128, 128:].rearrange("p (s m) -> p s m", s=NS - 1),
            )
            nc.scalar.dma_start(
                w[:, 1, :, :],
                d_w[128:256, :].rearrange("p (s m) -> p s m", s=NS),
            )
            nc.scalar.dma_start(
                w[:45, 2, :, :], d_w[256:DP, :].rearrange("p (s m) -> p s m", s=NS)
            )

            # PE p-state warm-up: the tensor engine runs at 1.2GHz until it
            # has been busy 3us.  Dummy matmuls on a zeroed tile during the
            # initial DMA window put the ramp behind us so every real matmul
            # runs at the full 2.4GHz from the first one.
            warm = wp.tile([128, 128], mm_dt, tag="warm", name="warm_t")
            nc.gpsimd.memset(warm[:, :], 0.0)
            pwarm = ps.tile([128, TS], F32, tag="ph0", name="ph_warm")
            for _ in range(2):
                nc.tensor.matmul(
                    pwarm[:, 0:128], lhsT=warm[:, :], rhs=warm[:, :],
                    start=True, stop=True,
                )

            # persistent ragged-gate tiles (one per pair): pad lanes memset
            # once to a scan-safe finite value, live lanes DMA'd per pair
            g2s = []
            for pr in range(BC // 2):
                g2 = g2p.tile([128, T], pl_dt, tag=f"g2_{pr}", name=f"g2_{pr}")
                nc.gpsimd.memset(g2[32:64, :], 0.5)
                nc.gpsimd.memset(g2[96:128, :], 0.5)
                g2s.append(g2)

            # A store dma_start holds the issuing engine's SEQ while its
            # data-ready sem is pending (cost model: waits precede
            # free(SEQ)).  On the ACT ring that starves the activations, so
            # stores ride the SP ring instead: a pair's stores are deferred
            # until after the NEXT pair's input loads have been issued (the
            # last pair's issue immediately -- nothing later rides SP).
            pending_stores: list = []

            def flush_stores():
                for ds, res_ap in pending_stores:
                    nc.sync.dma_start(ds, res_ap)
                pending_stores.clear()

            def chain(gs, z_ap, oneg_ap, stores, tsplit=1,
                      defer=True, res_pool=False, store_eng=None):
                """bneg=(g-1)z -> scan(-c) -> out = (-o)*(-c); stores is a
                list of (res_slice, dram_slice). tsplit>1 pipelines the chain
                in T-chunks (scan chained via `initial`) so the final store
                overlaps the rest -- used for the kernel-tail chain."""
                bneg = ep.tile([128, T], pl_dt, tag="bneg", name="bneg_t")
                cneg = ep.tile([128, T], c_dt, tag="c", name="cneg_t")
                res = ep.tile([128, T], out_dt, tag="res", name="res_t")
                gm1 = ep.tile([128, T], pl_dt, tag="gm1", name="gm1_t")
                tw = T // tsplit
                for h in range(tsplit):
                    hs = slice(h * tw, (h + 1) * tw)
                    # TS(4x) + TT(2x): cheapest legal bneg (scalar_tensor_
                    # tensor is not a Pool/1x-free op on this ISA)
                    nc.vector.tensor_scalar_add(gm1[:, hs], gs[:, hs], -1.0)
                    nc.vector.tensor_mul(bneg[:, hs], gm1[:, hs], z_ap[:, hs])
                    init = 0.0 if h == 0 else cneg[:, h * tw - 1 : h * tw]
                    nc.vector.tensor_tensor_scan(
                        cneg[:, hs], gs[:, hs], bneg[:, hs], init,
                        op0=mybir.AluOpType.mult, op1=mybir.AluOpType.add,
                    )
                    rsplit = 2 if (res_pool and store_eng is not None) else 1
                    rw = tw // rsplit
                    for r in range(rsplit):
                        rhs_ = slice(h * tw + r * rw, h * tw + (r + 1) * rw)
                        if res_pool:
                            # the final mult only feeds a deferred store, so
                            # the slow-but-idle GPSIMD engine absorbs it
                            # (half-T pieces so the stores pipeline behind it)
                            nc.gpsimd.tensor_mul(
                                res[:, rhs_], oneg_ap[:, rhs_], cneg[:, rhs_]
                            )
                        else:
                            nc.vector.tensor_mul(
                                res[:, rhs_], oneg_ap[:, rhs_], cneg[:, rhs_]
                            )
                        for rs, ds in stores:
                            if defer:
                                pending_stores.append(
                                    (ds[:, rhs_], res[rs[0] : rs[1], rhs_])
                                )
                            else:
                                (store_eng or nc.sync).dma_start(
                                    ds[:, rhs_], res[rs[0] : rs[1], rhs_]
                                )

            # Half-T psum tiles in a strict 4-deep round-robin (4 x 2 banks =
            # all of PSUM).  A unit's first GEMM then WAR-waits only on an
            # activation from a full unit earlier (long done) instead of the
            # previous unit's last.
            ph_ctr = [0]

            def psum_half():
                n = ph_ctr[0] % 4
                ph_ctr[0] += 1
                return ps.tile([128, T // 2], F32, tag=f"ph{n}", name=f"ph{n}")

            def gemm_half(p, xt, s, h):
                """Half-T matmul group for m-slice s: 3 k-chunks x 2
                T-blocks accumulating into the [128, 1024] psum p."""
                for ki, (k0, kn) in enumerate(KC):
                    for tb2 in range(NT // 2):
                        tb = h * (NT // 2) + tb2
                        nc.tensor.matmul(
                            p[:, bass.ts(tb2, TS)],
                            lhsT=w[:kn, ki, s, :],
                            rhs=xt[:kn, ki, bass.ts(tb, TS)],
                            start=ki == 0,
                            stop=ki == len(KC) - 1,
                        )

            for d_out in d_outs:
              for pair in range(BC // 2):
                b0, b1 = 2 * pair, 2 * pair + 1
                g2 = g2s[pair]
                xts = {}
                gts = {}
                for b in (b0, b1):
                    xt = xp.tile([128, 3, T], mm_dt, tag="x", name="xt_t")
                    # k0/k1 loaded in T-chunks so the first matmuls of each
                    # batch row start sooner (smaller first transfer)
                    nc.sync.dma_start(xt[:, 0, 0:512], d_x[b, 0:128, 0:512])
                    nc.sync.dma_start(xt[:, 0, 512:1024], d_x[b, 0:128, 512:1024])
                    nc.sync.dma_start(xt[:, 0, 1024:T], d_x[b, 0:128, 1024:T])
                    nc.sync.dma_start(xt[:, 1, 0:1024], d_x[b, 128:256, 0:1024])
                    nc.sync.dma_start(xt[:, 1, 1024:T], d_x[b, 128:256, 1024:T])
                    nc.sync.dma_start(xt[:45, 2, :], d_x[b, 256:DP, :])
                    xts[b] = xt
                    gt = gp.tile([128, 2, T], pl_dt, tag="g", name="gt_t")
                    nc.sync.dma_start(gt[:, 0, :], d_g[b, 0:128, :])
                    nc.sync.dma_start(gt[:, 1, :], d_g[b, 128:256, :])
                    gts[b] = gt
                nc.sync.dma_start(g2[0:44, :], d_g[b0, 256:D, :])
                nc.sync.dma_start(g2[64:108, :], d_g[b1, 256:D, :])
                # previous pair's stores go out only now, behind this pair's
                # input loads, so their data-ready waits never stall a load
                flush_stores()
                defer = pair < BC // 2 - 1

                def do_j(b, j, tsplit=1):
                    m0 = 128 * j
                    z_j = zp.tile([128, T], pl_dt, tag="z", name="t_z")
                    oneg_j = zp.tile([128, T], pl_dt, tag="o", name="t_o")
                    for s, dst, sc in ((j, z_j, 1.0), (2 + j, oneg_j, -1.0)):
                        for h in range(2):
                            hs = slice(h * (T // 2), (h + 1) * (T // 2))
                            p = psum_half()
                            gemm_half(p, xts[b], s, h)
                            nc.scalar.activation(dst[:, hs], p[:, :], Tanh, scale=sc)
                    chain(
                        gts[b][:, j, :], z_j[:, :], oneg_j[:, :],
                        [((0, 128), d_out[b, m0 : m0 + 128, :])],
                        tsplit=tsplit, defer=defer, res_pool=tsplit == 1,
                    )

                def rag_gemm_acts(b, z2, oneg2, lanes):
                    """Ragged slice GEMM for row b + repack activations into
                    z2/oneg2 at partition base `lanes` (0 for b_even, 64 for
                    b_odd).  The psum has z at 0:44 and o at 64:108; the
                    zero weight-pad columns make partitions 44:64 / 108:128
                    exact zeros, so the 64-wide activations also initialize
                    the pad lanes (tanh(0)=0) every pair."""
                    ls = slice(lanes, lanes + 64)
                    for h in range(2):
                        hs = slice(h * (T // 2), (h + 1) * (T // 2))
                        p = psum_half()
                        gemm_half(p, xts[b], 4, h)
                        nc.scalar.activation(z2[ls, hs], p[0:64, :], Tanh, scale=1.0)
                        nc.scalar.activation(
                            oneg2[ls, hs], p[64:128, :], Tanh, scale=-1.0
                        )

                # dedicated tags: sharing "z"/"o" with the unit tiles would
                # make the tail unit's activations WAR-wait on the ragged
                # chain's reads
                z2 = zp.tile([128, T], pl_dt, tag="z2", name="t_z2")
                oneg2 = zp.tile([128, T], pl_dt, tag="o2", name="t_o2")

                def tail_half(b, j, h, st):
                    """One T-half of the kernel-tail unit: GEMMs + acts +
                    chain piece.  h0 runs early in the pair so only this
                    half-chain trails the final matmul; the scan is stitched
                    across the gap via its `initial` operand."""
                    m0 = 128 * j
                    hs = slice(h * (T // 2), (h + 1) * (T // 2))
                    for s, dst, sc in ((j, st["z"], 1.0), (2 + j, st["o"], -1.0)):
                        p = psum_half()
                        gemm_half(p, xts[b], s, h)
                        nc.scalar.activation(dst[:, hs], p[:, :], Tanh, scale=sc)
                    nc.vector.tensor_scalar_add(
                        st["gm1"][:, hs], gts[b][:, j, hs], -1.0
                    )
                    nc.vector.tensor_mul(
                        st["bneg"][:, hs], st["gm1"][:, hs], st["z"][:, hs]
                    )
                    init = 0.0 if h == 0 else st["c"][:, T // 2 - 1 : T // 2]
                    nc.vector.tensor_tensor_scan(
                        st["c"][:, hs], gts[b][:, j, hs], st["bneg"][:, hs], init,
                        op0=mybir.AluOpType.mult, op1=mybir.AluOpType.add,
                    )
                    nc.vector.tensor_mul(
                        st["res"][:, hs], st["o"][:, hs], st["c"][:, hs]
                    )
                    nc.sync.dma_start(
                        d_out[b, m0 : m0 + 128, hs], st["res"][:, hs]
                    )

                last = pair == BC // 2 - 1
                do_j(b0, 0)
                rag_gemm_acts(b0, z2, oneg2, 0)
                do_j(b0, 1)
                if last:
                    # spread the tail unit: its first half completes mid-pair
                    # so only a half-chain trails the very last matmul
                    st = {
                        "z": zp.tile([128, T], pl_dt, tag="z", name="t_zT"),
                        "o": zp.tile([128, T], pl_dt, tag="o", name="t_oT"),
                        "gm1": ep.tile([128, T], pl_dt, tag="gm1", name="gm1_T"),
                        "bneg": ep.tile([128, T], pl_dt, tag="bneg", name="bneg_T"),
                        "c": ep.tile([128, T], c_dt, tag="c", name="cneg_T"),
                        "res": ep.tile([128, T], out_dt, tag="res", name="res_T"),
                    }
                    tail_half(b1, 1, 0, st)
                do_j(b1, 0)
                rag_gemm_acts(b1, z2, oneg2, 64)
                # ragged recurrence: both rows of the pair in one 128-lane
                # chain (b0 at 0:44, b1 at 64:108)
                chain(
                    g2[:, :], z2[:, :], oneg2[:, :],
                    [((0, 44), d_out[b0, 256:D, :]),
                     ((64, 108), d_out[b1, 256:D, :])],
                    defer=defer, res_pool=True,
                    store_eng=nc.scalar if last else None,
                )
                if last:
                    tail_half(b1, 1, 1, st)
                else:
                    do_j(b1, 1)

    nc.compile()
    return nc


def kernel(gate_encoding, inputs_encoding, Wz, bz, Wo, bo):
    gate_encoding = np.asarray(gate_encoding, dtype=np.float32)
    inputs_encoding = np.asarray(inputs_encoding, dtype=np.float32)
    Wz = np.asarray(Wz, dtype=np.float32)
    bz = np.asarray(bz, dtype=np.float32)
    Wo = np.asarray(Wo, dtype=np.float32)
    bo = np.asarray(bo, dtype=np.float32)

    mm_np = np.float16 if CFG["mm16"] else np.float32
    pl_np = np.float16 if CFG["plane16"] else np.float32

    wzo = build_weights(Wz, bz, Wo, bo, mm_np)

    if "nc" not in _CACHE:
        _CACHE["nc"] = _build_program()
    nc = _CACHE["nc"]

    in_maps = []
    for cc in range(NCORES):
        xs = inputs_encoding[cc * BC : (cc + 1) * BC]  # [BC, T, D]
        gs = gate_encoding[cc * BC : (cc + 1) * BC]
        xt = np.empty((BC, DP, T), dtype=mm_np)
        xt[:, :D, :] = xs.transpose(0, 2, 1)
        xt[:, D, :] = 1.0
        gt = gs.transpose(0, 2, 1).astype(pl_np)
        in_maps.append({"xt": xt, "gt": gt, "wzo": wzo})

    res = run_bass_kernel_spmd(nc, in_maps, core_ids=list(range(NCORES)))

    out = np.empty((B, T, D), dtype=np.float32)
    for cc in range(NCORES):
        out[cc * BC : (cc + 1) * BC] = (
            res.results[cc]["outt"].transpose(0, 2, 1).astype(np.float32)
        )
    return out


def build_weights(Wz, bz, Wo, bo, mm_np):
    """Combined [DP, 5*128] operand: slices 0,1 = Wz.T cols 0:256, slices
    2,3 = Wo.T cols 0:256, slice 4 = [Wz.T 256:300 | 0 | Wo.T 256:300 | 0]
    (zero pads keep the ragged psum's unused partitions exactly 0). The
    ones-row (DP-1) carries the biases."""

    def aug(Wmat, bvec):
        a = np.empty((DP, D), dtype=np.float32)
        a[:D, :] = Wmat.T
        a[D, :] = bvec
        return a

    wz_aug = aug(Wz, bz)
    wo_aug = aug(Wo, bo)
    wzo = np.zeros((DP, NS * 128), dtype=np.float32)
    wzo[:, 0:256] = wz_aug[:, 0:256]
    wzo[:, 256:512] = wo_aug[:, 0:256]
    wzo[:, 512:556] = wz_aug[:, 256:300]
    wzo[:, 576:620] = wo_aug[:, 256:300]
    return wzo.astype(mm_np)


# revision 56
# speedup vs baseline: 1.0489x; 1.0489x over previous
"""MRU encoding kernel for Trainium2 (8 NeuronCores, batch-parallel).

Problem (B=32, T=2048, D=300):
    z = tanh(x @ Wz.T + bz); o = tanh(x @ Wo.T + bo)
    c_t = g_t*c_{t-1} + (1-g_t)*z_t   (c_{-1}=0, scan over T)
    out = o * c

Per-core (4 batch rows) layout is [channel, time]:
  - host pre-transposes x,g to [b, D, T]; x gets a ones-row (301) so the
    bias rides in the matmul contraction; the two weight matrices are fed
    as ONE combined [D+1, 5, 128] operand: slices 0,1 = Wz.T columns
    0:256, slices 2,3 = Wo.T columns 0:256, slice 4 = the ragged columns
    of BOTH weights ([Wz.T 256:300 | zeros | Wo.T 256:300 | zeros]) so
    one GEMM per (row, slice) covers them: 15 matmul groups per row
    instead of 18.
  - o is produced NEGATED via tanh(scale=-1): with bneg=(g-1)*z = -(1-g)z
    the hardware scan state=g*state+bneg yields -c, and (-o)*(-c) = o*c.
  - the whole T=2048 recurrence per channel is ONE tensor_tensor_scan
    DVE instruction per 128-channel tile (state kept fp32 by HW); the
    final res mult rides the otherwise-idle GPSIMD engine (its only
    consumer is a deferred store, so GPSIMD's latency is free).
  - psums are half-T [128,1024] tiles in a strict 4-deep round-robin so
    a GEMM's WAR wait lands on an activation from a full unit earlier.
  - the ragged slice-4 psum holds z at partitions 0:44 and o at 64:108;
    partition-shifted activations repack BOTH batch rows of a pair into
    one 128-lane z2/oneg2 pair (b_even at 0:44, b_odd at 64:108) so the
    ragged recurrence is one DVE chain per pair.  The zero weight-pad
    columns make the activations also rewrite the pad lanes (tanh(0)=0),
    keeping every lane finite without extra memsets.
  - input loads ride the SP HWDGE ring; chain stores also ride SP but are
    deferred past the next pair's loads (a pending store holds the
    issuing SEQ, so it must never sit ahead of latency-critical work);
    weights ride the ACT ring; a 2-matmul warm-up puts the PE p-state
    ramp before the first real GEMM.
"""

import numpy as np

import concourse.bass as bass
import concourse.mybir as mybir
import concourse.tile as tile
from concourse import bacc
from concourse.bass_utils import run_bass_kernel_spmd

B, T, D = 32, 2048, 300
NCORES = 8
BC = B // NCORES  # 4 batch rows per core
DP = D + 1  # ones-row at index 300 carries the bias
NS = 5  # combined-weight m-slices: z0 z1 o0 o1 ragged
TS = 512  # moving-operand max free dim
NT = T // TS
F32 = mybir.dt.float32
F32R = mybir.dt.float32r
F16 = mybir.dt.float16

KC = [(0, 128), (128, 128), (256, 45)]  # k-chunks (incl. ones row)

CFG = {"mm16": True, "plane16": True, "c16": True, "out16": True}

_CACHE: dict = {}

Tanh = mybir.ActivationFunctionType.Tanh


def _build_program(reps=1, bufs=None, cfg=None):
    c = dict(CFG)
    if cfg:
        c.update(cfg)
    mm_dt = F16 if c["mm16"] else F32R
    pl_dt = F16 if c["plane16"] else F32
    c_dt = F16 if c["c16"] else F32
    out_dt = F16 if c["out16"] else F32

    bf = {"xp": 2, "gp": 2, "zp": 2, "ep": 4, "ps": 1}
    if bufs:
        bf.update(bufs)

    nc = bacc.Bacc("TRN2", target_bir_lowering=False, debug=False, num_devices=NCORES)

    d_x = nc.dram_tensor("xt", [BC, DP, T], mm_dt, kind="ExternalInput").ap()
    d_g = nc.dram_tensor("gt", [BC, D, T], pl_dt, kind="ExternalInput").ap()
    d_w = nc.dram_tensor("wzo", [DP, NS * 128], mm_dt, kind="ExternalInput").ap()
    # replicas share ONE output tensor: keeps the PJRT buffer count (and its
    # per-call overhead) constant across reps so marginal timing is clean
    d_out0 = nc.dram_tensor("outt", [BC, D, T], out_dt, kind="ExternalOutput").ap()
    d_outs = [d_out0] * reps

    with tile.TileContext(nc) as tc:
        with (
            tc.tile_pool(name="wp", bufs=1) as wp,
            tc.tile_pool(name="g2p", bufs=1) as g2p,
            tc.tile_pool(name="xp", bufs=bf["xp"]) as xp,
            tc.tile_pool(name="gp", bufs=bf["gp"]) as gp,
            tc.tile_pool(name="zp", bufs=bf["zp"]) as zp,
            tc.tile_pool(name="ep", bufs=bf["ep"]) as ep,
            tc.tile_pool(name="ps", bufs=bf["ps"], space="PSUM") as ps,
        ):
            # weights ride the scalar ring so they don't delay the first x
            # load; the first matmul's chunk (k-chunk 0, slice 0) goes out
            # alone so the opening GEMM isn't gated on the bulk transfer
            w = wp.tile([128, 3, NS, 128], mm_dt, tag="w", name="w_t")
            nc.scalar.dma_start(w[:, 0, 0, :], d_w[0:128, 0:128])
            nc.scalar.dma_start(
                w[:, 0, 1:NS, :],
                d_w[0:128, 128:].rearrange("p (s m) -> p s m", s=NS - 1),
            )
            nc.scalar.dma_start(
                w[:, 1, :, :],
                d_w[128:256, :].rearrange("p (s m) -> p s m", s=NS),
            )
            nc.scalar.dma_start(
                w[:45, 2, :, :], d_w[256:DP, :].rearrange("p (s m) -> p s m", s=NS)
            )

            # PE p-state warm-up: the tensor engine runs at 1.2GHz until it
            # has been busy 3us.  Dummy matmuls on a zeroed tile during the
            # initial DMA window put the ramp behind us so every real matmul
            # runs at the full 2.4GHz from the first one.
            warm = wp.tile([128, 128], mm_dt, tag="warm", name="warm_t")
            nc.gpsimd.memset(warm[:, :], 0.0)
            pwarm = ps.tile([128, TS], F32, tag="ph0", name="ph_warm")
            for _ in range(2):
                nc.tensor.matmul(
                    pwarm[:, 0:128], lhsT=warm[:, :], rhs=warm[:, :],
                    start=True, stop=True,
                )

            # persistent ragged-gate tiles (one per pair): pad lanes memset
            # once to a scan-safe finite value, live lanes DMA'd per pair
            g2s = []
            for pr in range(BC // 2):
                g2 = g2p.tile([128, T], pl_dt, tag=f"g2_{pr}", name=f"g2_{pr}")
                nc.gpsimd.memset(g2[32:64, :], 0.5)
                nc.gpsimd.memset(g2[96:128, :], 0.5)
                g2s.append(g2)

            # A store dma_start holds the issuing engine's SEQ while its
            # data-ready sem is pending (cost model: waits precede
            # free(SEQ)).  On the ACT ring that starves the activations, so
            # stores ride the SP ring instead: a pair's stores are deferred
            # until after the NEXT pair's input loads have been issued (the
            # last pair's issue immediately -- nothing later rides SP).
            pending_stores: list = []

            def flush_stores():
                for ds, res_ap in pending_stores:
                    nc.sync.dma_start(ds, res_ap)
                pending_stores.clear()

            def chain(gs, z_ap, oneg_ap, stores, tsplit=1,
                      defer=True, res_pool=False, store_eng=None):
                """bneg=(g-1)z -> scan(-c) -> out = (-o)*(-c); stores is a
                list of (res_slice, dram_slice). tsplit>1 pipelines the chain
                in T-chunks (scan chained via `initial`) so the final store
                overlaps the rest -- used for the kernel-tail chain."""
                bneg = ep.tile([128, T], pl_dt, tag="bneg", name="bneg_t")
                cneg = ep.tile([128, T], c_dt, tag="c", name="cneg_t")
                res = ep.tile([128, T], out_dt, tag="res", name="res_t")
                gm1 = ep.tile([128, T], pl_dt, tag="gm1", name="gm1_t")
                tw = T // tsplit
                for h in range(tsplit):
                    hs = slice(h * tw, (h + 1) * tw)
                    # TS(4x) + TT(2x): cheapest legal bneg (scalar_tensor_
                    # tensor is not a Pool/1x-free op on this ISA)
                    nc.vector.tensor_scalar_add(gm1[:, hs], gs[:, hs], -1.0)
                    nc.vector.tensor_mul(bneg[:, hs], gm1[:, hs], z_ap[:, hs])
                    init = 0.0 if h == 0 else cneg[:, h * tw - 1 : h * tw]
                    nc.vector.tensor_tensor_scan(
                        cneg[:, hs], gs[:, hs], bneg[:, hs], init,
                        op0=mybir.AluOpType.mult, op1=mybir.AluOpType.add,
                    )
                    rsplit = 2 if (res_pool and store_eng is not None) else 1
                    rw = tw // rsplit
                    for r in range(rsplit):
                        rhs_ = slice(h * tw + r * rw, h * tw + (r + 1) * rw)
                        if res_pool:
                            # the final mult only feeds a deferred store, so
                            # the slow-but-idle GPSIMD engine absorbs it
                            # (half-T pieces so the stores pipeline behind it)
                            nc.gpsimd.tensor_mul(
                                res[:, rhs_], oneg_ap[:, rhs_], cneg[:, rhs_]
                            )
                        else:
                            nc.vector.tensor_mul(
                                res[:, rhs_], oneg_ap[:, rhs_], cneg[:, rhs_]
                            )
                        for rs, ds in stores:
                            if defer:
                                pending_stores.append(
                                    (ds[:, rhs_], res[rs[0] : rs[1], rhs_])
                                )
                            else:
                                (store_eng or nc.sync).dma_start(
                                    ds[:, rhs_], res[rs[0] : rs[1], rhs_]
                                )

            # Half-T psum tiles in a strict 4-deep round-robin (4 x 2 banks =
            # all of PSUM).  A unit's first GEMM then WAR-waits only on an
            # activation from a full unit earlier (long done) instead of the
            # previous unit's last.
            ph_ctr = [0]

            def psum_half():
                n = ph_ctr[0] % 4
                ph_ctr[0] += 1
                return ps.tile([128, T // 2], F32, tag=f"ph{n}", name=f"ph{n}")

            def gemm_half(p, xt, s, h):
                """Half-T matmul group for m-slice s: 3 k-chunks x 2
                T-blocks accumulating into the [128, 1024] psum p."""
                for ki, (k0, kn) in enumerate(KC):
                    for tb2 in range(NT // 2):
                        tb = h * (NT // 2) + tb2
                        nc.tensor.matmul(
                            p[:, bass.ts(tb2, TS)],
                            lhsT=w[:kn, ki, s, :],
                            rhs=xt[:kn, ki, bass.ts(tb, TS)],
                            start=ki == 0,
                            stop=ki == len(KC) - 1,
                        )

            for d_out in d_outs:
              for pair in range(BC // 2):
                b0, b1 = 2 * pair, 2 * pair + 1
                g2 = g2s[pair]
                xts = {}
                gts = {}
                for b in (b0, b1):
                    xt = xp.tile([128, 3, T], mm_dt, tag="x", name="xt_t")
                    # k0/k1 loaded in T-chunks so the first matmuls of each
                    # batch row start sooner (smaller first transfer)
                    nc.sync.dma_start(xt[:, 0, 0:512], d_x[b, 0:128, 0:512])
                    nc.sync.dma_start(xt[:, 0, 512:1024], d_x[b, 0:128, 512:1024])
                    nc.sync.dma_start(xt[:, 0, 1024:T], d_x[b, 0:128, 1024:T])
                    nc.sync.dma_start(xt[:, 1, 0:1024], d_x[b, 128:256, 0:1024])
                    nc.sync.dma_start(xt[:, 1, 1024:T], d_x[b, 128:256, 1024:T])
                    nc.sync.dma_start(xt[:45, 2, :], d_x[b, 256:DP, :])
                    xts[b] = xt
                    gt = gp.tile([128, 2, T], pl_dt, tag="g", name="gt_t")
                    nc.sync.dma_start(gt[:, 0, :], d_g[b, 0:128, :])
                    nc.sync.dma_start(gt[:, 1, :], d_g[b, 128:256, :])
                    gts[b] = gt
                nc.sync.dma_start(g2[0:44, :], d_g[b0, 256:D, :])
                nc.sync.dma_start(g2[64:108, :], d_g[b1, 256:D, :])
                # previous pair's stores go out only now, behind this pair's
                # input loads, so their data-ready waits never stall a load
                flush_stores()
                defer = pair < BC // 2 - 1

                def do_j(b, j, tsplit=1):
                    m0 = 128 * j
                    z_j = zp.tile([128, T], pl_dt, tag="z", name="t_z")
                    oneg_j = zp.tile([128, T], pl_dt, tag="o", name="t_o")
                    for s, dst, sc in ((j, z_j, 1.0), (2 + j, oneg_j, -1.0)):
                        for h in range(2):
                            hs = slice(h * (T // 2), (h + 1) * (T // 2))
                            p = psum_half()
                            gemm_half(p, xts[b], s, h)
                            nc.scalar.activation(dst[:, hs], p[:, :], Tanh, scale=sc)
                    chain(
                        gts[b][:, j, :], z_j[:, :], oneg_j[:, :],
                        [((0, 128), d_out[b, m0 : m0 + 128, :])],
                        tsplit=tsplit, defer=defer, res_pool=tsplit == 1,
                    )

                def rag_gemm_acts(b, z2, oneg2, lanes):
                    """Ragged slice GEMM for row b + repack activations into
                    z2/oneg2 at partition base `lanes` (0 for b_even, 64 for
                    b_odd).  The psum has z at 0:44 and o at 64:108; the
                    zero weight-pad columns make partitions 44:64 / 108:128
                    exact zeros, so the 64-wide activations also initialize
                    the pad lanes (tanh(0)=0) every pair."""
                    ls = slice(lanes, lanes + 64)
                    for h in range(2):
                        hs = slice(h * (T // 2), (h + 1) * (T // 2))
                        p = psum_half()
                        gemm_half(p, xts[b], 4, h)
                        nc.scalar.activation(z2[ls, hs], p[0:64, :], Tanh, scale=1.0)
                        nc.scalar.activation(
                            oneg2[ls, hs], p[64:128, :], Tanh, scale=-1.0
                        )

                # dedicated tags: sharing "z"/"o" with the unit tiles would
                # make the tail unit's activations WAR-wait on the ragged
                # chain's reads
                z2 = zp.tile([128, T], pl_dt, tag="z2", name="t_z2")
                oneg2 = zp.tile([128, T], pl_dt, tag="o2", name="t_o2")

                last = pair == BC // 2 - 1
                do_j(b0, 0)
                rag_gemm_acts(b0, z2, oneg2, 0)
                do_j(b0, 1)
                do_j(b1, 0)
                rag_gemm_acts(b1, z2, oneg2, 64)
                # ragged recurrence: both rows of the pair in one 128-lane
                # chain (b0 at 0:44, b1 at 64:108)
                chain(
                    g2[:, :], z2[:, :], oneg2[:, :],
                    [((0, 44), d_out[b0, 256:D, :]),
                     ((64, 108), d_out[b1, 256:D, :])],
                    defer=defer, res_pool=True,
                    store_eng=nc.scalar if last else None,
                )
                # the kernel's very last chain is split in T-halves so its
                # scan/mul/store pipeline instead of dangling serially
                do_j(b1, 1, tsplit=2)

    nc.compile()
    return nc


def kernel(gate_encoding, inputs_encoding, Wz, bz, Wo, bo):
    gate_encoding = np.asarray(gate_encoding, dtype=np.float32)
    inputs_encoding = np.asarray(inputs_encoding, dtype=np.float32)
    Wz = np.asarray(Wz, dtype=np.float32)
    bz = np.asarray(bz, dtype=np.float32)
    Wo = np.asarray(Wo, dtype=np.float32)
    bo = np.asarray(bo, dtype=np.float32)

    mm_np = np.float16 if CFG["mm16"] else np.float32
    pl_np = np.float16 if CFG["plane16"] else np.float32

    wzo = build_weights(Wz, bz, Wo, bo, mm_np)

    if "nc" not in _CACHE:
        _CACHE["nc"] = _build_program()
    nc = _CACHE["nc"]

    in_maps = []
    for cc in range(NCORES):
        xs = inputs_encoding[cc * BC : (cc + 1) * BC]  # [BC, T, D]
        gs = gate_encoding[cc * BC : (cc + 1) * BC]
        xt = np.empty((BC, DP, T), dtype=mm_np)
        xt[:, :D, :] = xs.transpose(0, 2, 1)
        xt[:, D, :] = 1.0
        gt = gs.transpose(0, 2, 1).astype(pl_np)
        in_maps.append({"xt": xt, "gt": gt, "wzo": wzo})

    res = run_bass_kernel_spmd(nc, in_maps, core_ids=list(range(NCORES)))

    out = np.empty((B, T, D), dtype=np.float32)
    for cc in range(NCORES):
        out[cc * BC : (cc + 1) * BC] = (
            res.results[cc]["outt"].transpose(0, 2, 1).astype(np.float32)
        )
    return out


def build_weights(Wz, bz, Wo, bo, mm_np):
    """Combined [DP, 5*128] operand: slices 0,1 = Wz.T cols 0:256, slices
    2,3 = Wo.T cols 0:256, slice 4 = [Wz.T 256:300 | 0 | Wo.T 256:300 | 0]
    (zero pads keep the ragged psum's unused partitions exactly 0). The
    ones-row (DP-1) carries the biases."""

    def aug(Wmat, bvec):
        a = np.empty((DP, D), dtype=np.float32)
        a[:D, :] = Wmat.T
        a[D, :] = bvec
        return a

    wz_aug = aug(Wz, bz)
    wo_aug = aug(Wo, bo)
    wzo = np.zeros((DP, NS * 128), dtype=np.float32)
    wzo[:, 0:256] = wz_aug[:, 0:256]
    wzo[:, 256:512] = wo_aug[:, 0:256]
    wzo[:, 512:556] = wz_aug[:, 256:300]
    wzo[:, 576:620] = wo_aug[:, 256:300]
    return wzo.astype(mm_np)


# revision 73
# speedup vs baseline: 1.0847x; 1.0341x over previous
"""MRU encoding kernel for Trainium2 (8 NeuronCores, batch-parallel).

Problem (B=32, T=2048, D=300):
    z = tanh(x @ Wz.T + bz); o = tanh(x @ Wo.T + bo)
    c_t = g_t*c_{t-1} + (1-g_t)*z_t   (c_{-1}=0, scan over T)
    out = o * c

Per-core (4 batch rows) layout is [channel, time]:
  - host pre-transposes x,g to [b, D, T]; x gets a ones-row (301) so the
    bias rides in the matmul contraction; the two weight matrices are fed
    as ONE combined [D+1, 5, 128] operand: slices 0,1 = Wz.T columns
    0:256, slices 2,3 = Wo.T columns 0:256, slice 4 = the ragged columns
    of BOTH weights ([Wz.T 256:300 | zeros | Wo.T 256:300 | zeros]) so
    one GEMM per (row, slice) covers them: 15 matmul groups per row
    instead of 18.
  - o is produced NEGATED via tanh(scale=-1): with bneg=(g-1)*z = -(1-g)z
    the hardware scan state=g*state+bneg yields -c, and (-o)*(-c) = o*c.
  - the whole T=2048 recurrence per channel is ONE tensor_tensor_scan
    DVE instruction per 128-channel tile (state kept fp32 by HW); the
    final res mult rides the otherwise-idle GPSIMD engine (its only
    consumer is a deferred store, so GPSIMD's latency is free).
  - psums are half-T [128,1024] tiles in a strict 4-deep round-robin so
    a GEMM's WAR wait lands on an activation from a full unit earlier.
  - the ragged slice-4 psum holds z at partitions 0:44 and -o at 64:108
    (the host negates Wo's ragged columns), so ONE full-width activation
    per psum half covers both: 2 ragged acts per row instead of 4, which
    takes the act-oversubscribed ragged units off the ACT critical path.
    A cheap SBUF->SBUF DMA realigns the -o lanes to partition base 0
    (the verifier requires equal base partitions for TensorTensor), and
    each row runs its own 44-lane recurrence chain.
  - input loads ride the SP HWDGE ring; chain stores also ride SP but are
    deferred past the next pair's loads (a pending store holds the
    issuing SEQ, so it must never sit ahead of latency-critical work);
    weights ride the ACT ring; a 2-matmul warm-up puts the PE p-state
    ramp before the first real GEMM.
"""

import numpy as np

import concourse.bass as bass
import concourse.mybir as mybir
import concourse.tile as tile
from concourse import bacc
from concourse.bass_utils import run_bass_kernel_spmd

B, T, D = 32, 2048, 300
NCORES = 8
BC = B // NCORES  # 4 batch rows per core
DP = D + 1  # ones-row at index 300 carries the bias
NS = 5  # combined-weight m-slices: z0 z1 o0 o1 ragged
TS = 512  # moving-operand max free dim
NT = T // TS
F32 = mybir.dt.float32
F32R = mybir.dt.float32r
F16 = mybir.dt.float16

KC = [(0, 128), (128, 128), (256, 45)]  # k-chunks (incl. ones row)

CFG = {"mm16": True, "plane16": True, "c16": True, "out16": True}

_CACHE: dict = {}

Tanh = mybir.ActivationFunctionType.Tanh


def _build_program(reps=1, bufs=None, cfg=None):
    c = dict(CFG)
    if cfg:
        c.update(cfg)
    mm_dt = F16 if c["mm16"] else F32R
    pl_dt = F16 if c["plane16"] else F32
    c_dt = F16 if c["c16"] else F32
    out_dt = F16 if c["out16"] else F32

    bf = {"xp": 2, "gp": 2, "zp": 2, "ep": 4, "ps": 1}
    if bufs:
        bf.update(bufs)

    nc = bacc.Bacc("TRN2", target_bir_lowering=False, debug=False, num_devices=NCORES)

    d_x = nc.dram_tensor("xt", [BC, DP, T], mm_dt, kind="ExternalInput").ap()
    d_g = nc.dram_tensor("gt", [BC, D, T], pl_dt, kind="ExternalInput").ap()
    d_w = nc.dram_tensor("wzo", [DP, NS * 128], mm_dt, kind="ExternalInput").ap()
    # replicas share ONE output tensor: keeps the PJRT buffer count (and its
    # per-call overhead) constant across reps so marginal timing is clean
    d_out0 = nc.dram_tensor("outt", [BC, D, T], out_dt, kind="ExternalOutput").ap()
    d_outs = [d_out0] * reps

    with tile.TileContext(nc) as tc:
        with (
            tc.tile_pool(name="wp", bufs=1) as wp,
            tc.tile_pool(name="xp", bufs=bf["xp"]) as xp,
            tc.tile_pool(name="gp", bufs=bf["gp"]) as gp,
            tc.tile_pool(name="zp", bufs=bf["zp"]) as zp,
            tc.tile_pool(name="ep", bufs=bf["ep"]) as ep,
            tc.tile_pool(name="ps", bufs=bf["ps"], space="PSUM") as ps,
        ):
            # weights ride the scalar ring so they don't delay the first x
            # load; the first matmul's chunk (k-chunk 0, slice 0) goes out
            # alone so the opening GEMM isn't gated on the bulk transfer
            w = wp.tile([128, 3, NS, 128], mm_dt, tag="w", name="w_t")
            nc.scalar.dma_start(w[:, 0, 0, :], d_w[0:128, 0:128])
            nc.scalar.dma_start(
                w[:, 0, 1:NS, :],
                d_w[0:128, 128:].rearrange("p (s m) -> p s m", s=NS - 1),
            )
            nc.scalar.dma_start(
                w[:, 1, :, :],
                d_w[128:256, :].rearrange("p (s m) -> p s m", s=NS),
            )
            nc.scalar.dma_start(
                w[:45, 2, :, :], d_w[256:DP, :].rearrange("p (s m) -> p s m", s=NS)
            )

            # PE p-state warm-up: the tensor engine runs at 1.2GHz until it
            # has been busy 3us.  Dummy matmuls on a zeroed tile during the
            # initial DMA window put the ramp behind us so every real matmul
            # runs at the full 2.4GHz from the first one.
            warm = wp.tile([128, 128], mm_dt, tag="warm", name="warm_t")
            nc.gpsimd.memset(warm[:, :], 0.0)
            pwarm = ps.tile([128, TS], F32, tag="ph0", name="ph_warm")
            for _ in range(2):
                nc.tensor.matmul(
                    pwarm[:, 0:128], lhsT=warm[:, :], rhs=warm[:, :],
                    start=True, stop=True,
                )



            # A store dma_start holds the issuing engine's SEQ while its
            # data-ready sem is pending (cost model: waits precede
            # free(SEQ)).  On the ACT ring that starves the activations, so
            # stores ride the SP ring instead: a pair's stores are deferred
            # until after the NEXT pair's input loads have been issued (the
            # last pair's issue immediately -- nothing later rides SP).
            pending_stores: list = []

            def flush_stores():
                for ds, res_ap in pending_stores:
                    nc.sync.dma_start(ds, res_ap)
                pending_stores.clear()

            def chain(gs, z_ap, oneg_ap, stores, tsplit=1,
                      defer=True, res_pool=False, store_eng=None):
                """bneg=(g-1)z -> scan(-c) -> out = (-o)*(-c); stores is a
                list of (res_slice, dram_slice). tsplit>1 pipelines the chain
                in T-chunks (scan chained via `initial`) so the final store
                overlaps the rest -- used for the kernel-tail chain."""
                bneg = ep.tile([128, T], pl_dt, tag="bneg", name="bneg_t")
                cneg = ep.tile([128, T], c_dt, tag="c", name="cneg_t")
                res = ep.tile([128, T], out_dt, tag="res", name="res_t")
                gm1 = ep.tile([128, T], pl_dt, tag="gm1", name="gm1_t")
                tw = T // tsplit
                for h in range(tsplit):
                    hs = slice(h * tw, (h + 1) * tw)
                    # TS(4x) + TT(2x): cheapest legal bneg (scalar_tensor_
                    # tensor is not a Pool/1x-free op on this ISA)
                    nc.vector.tensor_scalar_add(gm1[:, hs], gs[:, hs], -1.0)
                    nc.vector.tensor_mul(bneg[:, hs], gm1[:, hs], z_ap[:, hs])
                    init = 0.0 if h == 0 else cneg[:, h * tw - 1 : h * tw]
                    nc.vector.tensor_tensor_scan(
                        cneg[:, hs], gs[:, hs], bneg[:, hs], init,
                        op0=mybir.AluOpType.mult, op1=mybir.AluOpType.add,
                    )
                    rsplit = 2 if (res_pool and store_eng is not None) else 1
                    rw = tw // rsplit
                    for r in range(rsplit):
                        rhs_ = slice(h * tw + r * rw, h * tw + (r + 1) * rw)
                        if res_pool:
                            # the final mult only feeds a deferred store, so
                            # the slow-but-idle GPSIMD engine absorbs it
                            # (half-T pieces so the stores pipeline behind it)
                            nc.gpsimd.tensor_mul(
                                res[:, rhs_], oneg_ap[:, rhs_], cneg[:, rhs_]
                            )
                        else:
                            nc.vector.tensor_mul(
                                res[:, rhs_], oneg_ap[:, rhs_], cneg[:, rhs_]
                            )
                        for rs, ds in stores:
                            if defer:
                                pending_stores.append(
                                    (ds[:, rhs_], res[rs[0] : rs[1], rhs_])
                                )
                            else:
                                (store_eng or nc.sync).dma_start(
                                    ds[:, rhs_], res[rs[0] : rs[1], rhs_]
                                )

            # Half-T psum tiles in a strict 4-deep round-robin (4 x 2 banks =
            # all of PSUM).  A unit's first GEMM then WAR-waits only on an
            # activation from a full unit earlier (long done) instead of the
            # previous unit's last.
            ph_ctr = [0]

            def psum_half():
                n = ph_ctr[0] % 4
                ph_ctr[0] += 1
                return ps.tile([128, T // 2], F32, tag=f"ph{n}", name=f"ph{n}")

            def gemm_half(p, xt, s, h):
                """Half-T matmul group for m-slice s: 3 k-chunks x 2
                T-blocks accumulating into the [128, 1024] psum p."""
                for ki, (k0, kn) in enumerate(KC):
                    for tb2 in range(NT // 2):
                        tb = h * (NT // 2) + tb2
                        nc.tensor.matmul(
                            p[:, bass.ts(tb2, TS)],
                            lhsT=w[:kn, ki, s, :],
                            rhs=xt[:kn, ki, bass.ts(tb, TS)],
                            start=ki == 0,
                            stop=ki == len(KC) - 1,
                        )

            for d_out in d_outs:
              for pair in range(BC // 2):
                b0, b1 = 2 * pair, 2 * pair + 1
                xts = {}
                gts = {}
                g2s = {}
                for b in (b0, b1):
                    xt = xp.tile([128, 3, T], mm_dt, tag="x", name="xt_t")
                    # k0/k1 loaded in T-chunks so the first matmuls of each
                    # batch row start sooner (smaller first transfer)
                    nc.sync.dma_start(xt[:, 0, 0:512], d_x[b, 0:128, 0:512])
                    nc.sync.dma_start(xt[:, 0, 512:1024], d_x[b, 0:128, 512:1024])
                    nc.sync.dma_start(xt[:, 0, 1024:T], d_x[b, 0:128, 1024:T])
                    nc.sync.dma_start(xt[:, 1, 0:1024], d_x[b, 128:256, 0:1024])
                    nc.sync.dma_start(xt[:, 1, 1024:T], d_x[b, 128:256, 1024:T])
                    nc.sync.dma_start(xt[:45, 2, :], d_x[b, 256:DP, :])
                    xts[b] = xt
                    gt = gp.tile([128, 2, T], pl_dt, tag="g", name="gt_t")
                    nc.sync.dma_start(gt[:, 0, :], d_g[b, 0:128, :])
                    nc.sync.dma_start(gt[:, 1, :], d_g[b, 128:256, :])
                    gts[b] = gt
                    g2r = gp.tile([64, T], pl_dt, tag="g2r", name="g2r_t")
                    nc.sync.dma_start(g2r[0:44, :], d_g[b, 256:D, :])
                    g2s[b] = g2r
                # previous pair's stores go out only now, behind this pair's
                # input loads, so their data-ready waits never stall a load
                flush_stores()
                defer = pair < BC // 2 - 1

                def do_j(b, j, tsplit=1):
                    m0 = 128 * j
                    z_j = zp.tile([128, T], pl_dt, tag="z", name="t_z")
                    oneg_j = zp.tile([128, T], pl_dt, tag="o", name="t_o")
                    for s, dst, sc in ((j, z_j, 1.0), (2 + j, oneg_j, -1.0)):
                        for h in range(2):
                            hs = slice(h * (T // 2), (h + 1) * (T // 2))
                            p = psum_half()
                            gemm_half(p, xts[b], s, h)
                            nc.scalar.activation(dst[:, hs], p[:, :], Tanh, scale=sc)
                    chain(
                        gts[b][:, j, :], z_j[:, :], oneg_j[:, :],
                        [((0, 128), d_out[b, m0 : m0 + 128, :])],
                        tsplit=tsplit, defer=defer, res_pool=tsplit == 1,
                    )

                last = pair == BC // 2 - 1

                def rag_row(b, store_act=False):
                    """Ragged slice for one row: GEMM + ONE full-width act
                    per half (z lands at 0:44, -o at 64:108 -- the host
                    negates Wo's ragged columns so one scale works for both),
                    a DMA realign of the -o lanes to base 0 (the same-base
                    TensorTensor rule forbids mixed-base operands), then a
                    44-lane recurrence chain."""
                    zo = zp.tile([128, T], pl_dt, tag="zo", name="t_zo")
                    for h in range(2):
                        hs = slice(h * (T // 2), (h + 1) * (T // 2))
                        p = psum_half()
                        gemm_half(p, xts[b], 4, h)
                        nc.scalar.activation(zo[:, hs], p[:, :], Tanh, scale=1.0)
                    # realign -o to partition base 0; rides the ACT ring
                    # directly behind its source acts so the wait is nil
                    ocp = zp.tile([64, T], pl_dt, tag="ocp", name="t_ocp")
                    nc.scalar.dma_start(ocp[0:44, :], zo[64:108, :])
                    g2r = g2s[b]
                    gm1r = ep.tile([64, T], pl_dt, tag="rgm1", name="rgm1_t", bufs=2)
                    bnegr = ep.tile([64, T], pl_dt, tag="rbneg", name="rbneg_t", bufs=2)
                    cnegr = ep.tile([64, T], c_dt, tag="rc", name="rcneg_t", bufs=2)
                    resr = ep.tile([64, T], out_dt, tag="rres", name="rres_t", bufs=2)
                    nc.vector.tensor_scalar_add(gm1r[0:44, :], g2r[0:44, :], -1.0)
                    nc.vector.tensor_mul(bnegr[0:44, :], gm1r[0:44, :], zo[0:44, :])
                    nc.vector.tensor_tensor_scan(
                        cnegr[0:44, :], g2r[0:44, :], bnegr[0:44, :], 0.0,
                        op0=mybir.AluOpType.mult, op1=mybir.AluOpType.add,
                    )
                    if store_act:
                        # kernel-tail row: defer the res to the caller so its
                        # stores are emitted after the tail unit's acts (a
                        # waiting store would hold the ACT SEQ against them)
                        return b, ocp, cnegr, resr
                    nc.gpsimd.tensor_mul(resr[0:44, :], ocp[0:44, :], cnegr[0:44, :])
                    if defer:
                        pending_stores.append(
                            (d_out[b, 256:D, :], resr[0:44, :])
                        )
                    else:
                        nc.sync.dma_start(d_out[b, 256:D, :], resr[0:44, :])
                    return None

                do_j(b0, 0)
                rag_row(b0)
                do_j(b0, 1)
                do_j(b1, 0)
                tail_rag = rag_row(b1, store_act=last)
                # the kernel's very last chain is split in T-halves so its
                # scan/mul/store pipeline instead of dangling serially
                do_j(b1, 1, tsplit=2)
                if tail_rag is not None:
                    rb, ocp, cnegr, resr = tail_rag
                    for r in range(2):
                        rs_ = slice(r * (T // 2), (r + 1) * (T // 2))
                        nc.gpsimd.tensor_mul(
                            resr[0:44, rs_], ocp[0:44, rs_], cnegr[0:44, rs_]
                        )
                        nc.scalar.dma_start(
                            d_out[rb, 256:D, rs_], resr[0:44, rs_]
                        )

    nc.compile()
    return nc


def kernel(gate_encoding, inputs_encoding, Wz, bz, Wo, bo):
    gate_encoding = np.asarray(gate_encoding, dtype=np.float32)
    inputs_encoding = np.asarray(inputs_encoding, dtype=np.float32)
    Wz = np.asarray(Wz, dtype=np.float32)
    bz = np.asarray(bz, dtype=np.float32)
    Wo = np.asarray(Wo, dtype=np.float32)
    bo = np.asarray(bo, dtype=np.float32)

    mm_np = np.float16 if CFG["mm16"] else np.float32
    pl_np = np.float16 if CFG["plane16"] else np.float32

    wzo = build_weights(Wz, bz, Wo, bo, mm_np)

    if "nc" not in _CACHE:
        _CACHE["nc"] = _build_program()
    nc = _CACHE["nc"]

    in_maps = []
    for cc in range(NCORES):
        xs = inputs_encoding[cc * BC : (cc + 1) * BC]  # [BC, T, D]
        gs = gate_encoding[cc * BC : (cc + 1) * BC]
        xt = np.empty((BC, DP, T), dtype=mm_np)
        xt[:, :D, :] = xs.transpose(0, 2, 1)
        xt[:, D, :] = 1.0
        gt = gs.transpose(0, 2, 1).astype(pl_np)
        in_maps.append({"xt": xt, "gt": gt, "wzo": wzo})

    res = run_bass_kernel_spmd(nc, in_maps, core_ids=list(range(NCORES)))

    out = np.empty((B, T, D), dtype=np.float32)
    for cc in range(NCORES):
        out[cc * BC : (cc + 1) * BC] = (
            res.results[cc]["outt"].transpose(0, 2, 1).astype(np.float32)
        )
    return out


def build_weights(Wz, bz, Wo, bo, mm_np):
    """Combined [DP, 5*128] operand: slices 0,1 = Wz.T cols 0:256, slices
    2,3 = Wo.T cols 0:256, slice 4 = [Wz.T 256:300 | 0 | Wo.T 256:300 | 0]
    (zero pads keep the ragged psum's unused partitions exactly 0). The
    ones-row (DP-1) carries the biases."""

    def aug(Wmat, bvec):
        a = np.empty((DP, D), dtype=np.float32)
        a[:D, :] = Wmat.T
        a[D, :] = bvec
        return a

    wz_aug = aug(Wz, bz)
    wo_aug = aug(Wo, bo)
    wzo = np.zeros((DP, NS * 128), dtype=np.float32)
    wzo[:, 0:256] = wz_aug[:, 0:256]
    wzo[:, 256:512] = wo_aug[:, 0:256]
    wzo[:, 512:556] = wz_aug[:, 256:300]
    # negated: the ragged activation then uses one scale (+1) for both
    # the z lanes and the (-o) lanes
    wzo[:, 576:620] = -wo_aug[:, 256:300]
    return wzo.astype(mm_np)
